# revision 1
# baseline (speedup 1.0000x reference)
"""AlibiTransformerLayer on 8 TRN2 NeuronCores (Bass/Tile, SPMD).

Sharding:
  - Tokens: core c owns 512 tokens: batch c//4, slice [512*(c%4), 512*(c%4)+512).
    LayerNorms, QKV, out-projection, FFN and residuals run token-sharded.
  - Attention: head-sharded globally: core c handles heads {2c, 2c+1} for BOTH
    batches. AllToAll #1 redistributes q,k,v tokens->heads; AllToAll #2
    redistributes ctx heads->tokens.

Layout: feature-major on chip (features on partitions, tokens on free dim).
Host pre-folds LayerNorm affine into adjacent weights, pre-scales wq by
1/sqrt(hd), and precomputes column sums so projections run on the raw input
with an affine fix-up:  W'x_ln = W'(a*x) + (-mu*a)*colsum(W') + bias.
Softmax runs in scores^T orientation (keys on partitions): ALiBi bias
-(i+j)*2^-h is separable; -i*s cancels in softmax, -j*s is a per-partition
bias fused into the exp. The softmax denominator is folded into the probs@v
matmul: v SBUF tiles are [128, 8x65] with a ones-column appended to each
64-col (kt,head) group, so one matmul emits ctx rows 0-63 plus the
denominator in psum row 64. No max-subtraction is needed since the j=0
column always contributes an O(1) term.

Input staging is deduplicated: wo|w1|w2 (identical on all 8 cores) are staged
as a 1/8 shard per core and AllGather'd on device into a Shared DRAM buffer
while attention runs (the CC queue is idle between the v AllToAll and the ctx
AllToAll); x arrives once as fp32 and is cast to bf16 on device; uvb/cvb ride
as [1, D] rows and are partition-broadcast on device. Per-core staged input
bytes drop from ~27.5MB to ~10MB, and measured per-body time drops ~40% by
taking the FFN/out-proj weight DMA pressure off the compute phases.
"""

import numpy as np
import ml_dtypes
from contextlib import ExitStack

import concourse.bacc as bacc
import concourse.mybir as mybir
import concourse.tile as tile
from concourse.bass_utils import run_bass_kernel_spmd

FP32 = mybir.dt.float32
BF16 = mybir.dt.bfloat16
AF = mybir.ActivationFunctionType
ALU = mybir.AluOpType

N_CORES = 8
B, S, D = 2, 2048, 1024
NH, HD = 16, 64
DFF = 4096
EPS = 1e-5
T = 512            # tokens owned per core
NEG = -1e5         # causal mask add
KT = D // 128      # 8 feature k-tiles
NTT = T // 128     # 4 token tiles

_cache = {}


def _build(sim=False, phase_marks=None, reps=1, fake_inputs=False, skip=()):
    def _mark(name):
        if phase_marks is not None:
            phase_marks.append((name, _nc_for_marks.next_id()))
    nc = bacc.Bacc("TRN2", target_bir_lowering=False, debug=False,
                   enable_asserts=True, num_devices=N_CORES)
    _nc_for_marks = nc

    EXT = "Internal" if fake_inputs else "ExternalInput"
    d_xT = nc.dram_tensor("xT", [D, T], FP32, kind=EXT).ap()
    # pre-tiled weights
    d_wqk = nc.dram_tensor("wqkt", [16, 128, KT * 128], BF16, kind=EXT).ap()
    d_wv = nc.dram_tensor("wvt", [2, 128, KT * 512], BF16, kind=EXT).ap()
    # wo|w1|w2 are identical on every core: stage 1/8 per core, AllGather on
    # device (overlapped with attention) instead of staging 16MB x 8 copies.
    WSH = (KT * 128 * KT * 128) + 2 * (DFF * KT * 128)   # 9437184 elems
    d_wsh = nc.dram_tensor("wsh", [WSH // N_CORES], BF16, kind=EXT).ap()
    d_wshb = nc.dram_tensor("wshb", [WSH // N_CORES], BF16, kind="Internal").ap()
    d_wfull = nc.dram_tensor("wfull", [WSH], BF16, kind="Internal",
                             addr_space="Shared").ap()
    _NWO = KT * 128 * KT * 128
    _NW1 = DFF * KT * 128
    d_wo = d_wfull[0:_NWO].rearrange("(a p f) -> a p f", a=KT, p=128)
    d_w1 = d_wfull[_NWO:_NWO + _NW1].rearrange("(a p f) -> a p f", a=DFF // 128, p=128)
    d_w2 = d_wfull[_NWO + _NW1:].rearrange("(a p f) -> a p f", a=KT, p=128)
    # per-column constants (partition-major)
    d_uqk = nc.dram_tensor("uqk", [128, 16], FP32, kind="ExternalInput").ap()
    d_cqk = nc.dram_tensor("cqk", [128, 16], FP32, kind=EXT).ap()
    d_u1 = nc.dram_tensor("u1", [128, 32], FP32, kind=EXT).ap()
    d_uvb = nc.dram_tensor("uvb", [1, D], BF16, kind=EXT).ap()
    d_cvb = nc.dram_tensor("cvb", [1, D], BF16, kind=EXT).ap()
    d_bo = nc.dram_tensor("bo", [128, 8], FP32, kind=EXT).ap()
    d_b1 = nc.dram_tensor("b1", [128, 32], FP32, kind=EXT).ap()
    d_b2 = nc.dram_tensor("b2", [128, 8], FP32, kind=EXT).ap()
    d_alibi = nc.dram_tensor("alibi", [128, 32], FP32, kind=EXT).ap()
    d_masks = nc.dram_tensor("masks", [128, 2048], BF16, kind=EXT).ap()
    d_ident = nc.dram_tensor("ident", [128, 128], FP32, kind=EXT).ap()
    d_out = nc.dram_tensor("out", [D, T], FP32, kind="ExternalOutput").ap()

    def _body(tc, rp):
        with ExitStack() as ctx:
            pool_const = ctx.enter_context(tc.tile_pool(name="const" + rp, bufs=1))
            pool_rows = ctx.enter_context(tc.tile_pool(name="rows" + rp, bufs=1))
            pool_dram = ctx.enter_context(tc.tile_pool(name="dramp" + rp, bufs=1, space="DRAM"))
            pool_xf = ctx.enter_context(tc.tile_pool(name="xf" + rp, bufs=1))
            pool_s2 = ctx.enter_context(tc.tile_pool(name="s2" + rp, bufs=1))
            # LIFO-scoped pools (closed mid-program, innermost last-opened first)
            cm_w1p = tc.tile_pool(name="w1p" + rp, bufs=6); pool_w1p = cm_w1p.__enter__()
            cm_ctxf = tc.tile_pool(name="ctxf" + rp, bufs=1); pool_ctxf = cm_ctxf.__enter__()
            cm_wop = tc.tile_pool(name="wop" + rp, bufs=1); pool_wop = cm_wop.__enter__()
            cm_att = tc.tile_pool(name="attp" + rp, bufs=1); pool_att = cm_att.__enter__()
            cm_qkv = tc.tile_pool(name="qkvout" + rp, bufs=1); pool_qkv = cm_qkv.__enter__()
            cm_xb = tc.tile_pool(name="xbp" + rp, bufs=1); pool_xb = cm_xb.__enter__()

            nc.sync.dma_start(d_wshb[:], d_wsh[:])
            x_f = [pool_xf.tile([128, T], FP32, tag=f"xf{r}", name=f"xf{r}") for r in range(KT)]
            x_bf = [pool_xb.tile([128, T], BF16, tag=f"xb{r}", name=f"xb{r}") for r in range(KT)]
            for r in range(KT):
                nc.sync.dma_start(x_f[r][:], d_xT[128 * r:128 * (r + 1), :])
                nc.scalar.copy(x_bf[r][:], x_f[r][:])

            # ---- constants ----
            uqk = pool_const.tile([128, 16], FP32, name="uqk"); nc.sync.dma_start(uqk[:], d_uqk[:])
            cqk = pool_const.tile([128, 16], FP32, name="cqk"); nc.sync.dma_start(cqk[:], d_cqk[:])
            uvb_r = pool_const.tile([1, D], BF16, name="uvb_r"); nc.sync.dma_start(uvb_r[:], d_uvb[:])
            cvb_r = pool_const.tile([1, D], BF16, name="cvb_r"); nc.sync.dma_start(cvb_r[:], d_cvb[:])
            uvb = pool_const.tile([128, D], BF16, name="uvb")
            nc.gpsimd.partition_broadcast(uvb[:], uvb_r[:])
            cvb = pool_const.tile([128, D], BF16, name="cvb")
            nc.gpsimd.partition_broadcast(cvb[:], cvb_r[:])
            bo = pool_const.tile([128, 8], FP32, name="bo"); nc.sync.dma_start(bo[:], d_bo[:])
            u1 = pool_const.tile([128, 32], FP32, name="u1"); nc.sync.dma_start(u1[:], d_u1[:])
            b1 = pool_const.tile([128, 32], FP32, name="b1"); nc.sync.dma_start(b1[:], d_b1[:])
            b2 = pool_const.tile([128, 8], FP32, name="b2"); nc.sync.dma_start(b2[:], d_b2[:])
            alibi = pool_const.tile([128, 32], FP32, name="alibi"); nc.sync.dma_start(alibi[:], d_alibi[:])
            masks = pool_const.tile([128, 2048], BF16, name="masks"); nc.sync.dma_start(masks[:], d_masks[:])
            ident = pool_const.tile([128, 128], FP32, name="ident"); nc.sync.dma_start(ident[:], d_ident[:])
            ones_bf = pool_const.tile([128, 1], BF16, name="ones_bf"); nc.vector.memset(ones_bf[:], 1.0)
            # prime the ACT sqrt table while the engines are DMA-bound at startup,
            # so LN1's Sqrt doesn't pay the ~2.7us table load on its critical chain
            prime = pool_const.tile([1, 1], FP32, name="prime")
            nc.vector.memset(prime[:], 1.0)
            nc.scalar.activation(prime[:], prime[:], AF.Sqrt)

            # ---- LN stats helper ----
            def ln_stats(xbf_tiles, name, want_pm=False):
                out = {}
                with tc.tile_pool(name=f"{name}_tmp" + rp, bufs=2) as ptmp, \
                     tc.tile_pool(name=f"{name}_ps" + rp, bufs=1, space="PSUM") as pps:
                    s_ps = pps.tile([1, T], FP32, tag="s", name="s")
                    q_ps = pps.tile([1, T], FP32, tag="q", name="q")
                    for r in range(KT):
                        sq = ptmp.tile([128, T], BF16, tag="sq", name="sq")
                        nc.vector.tensor_mul(sq[:], xbf_tiles[r][:], xbf_tiles[r][:])
                        nc.tensor.matmul(s_ps[:], ones_bf[:], xbf_tiles[r][:],
                                         start=(r == 0), stop=(r == KT - 1))
                        nc.tensor.matmul(q_ps[:], ones_bf[:], sq[:],
                                         start=(r == 0), stop=(r == KT - 1))
                    mu = pool_rows.tile([1, T], FP32, tag="mu", name="mu")
                    nc.scalar.activation(mu[:], s_ps[:], AF.Copy, scale=1.0 / D)
                    m2 = pool_rows.tile([1, T], FP32, tag="m2", name="m2")
                    nc.scalar.activation(m2[:], q_ps[:], AF.Copy, scale=1.0 / D)
                    var = pool_rows.tile([1, T], FP32, tag="var", name="var")
                    nc.vector.scalar_tensor_tensor(var[:], mu[:], -1.0, mu[:],
                                                   op0=ALU.mult, op1=ALU.mult)
                    nc.vector.scalar_tensor_tensor(var[:], m2[:], float(EPS), var[:],
                                                   op0=ALU.add, op1=ALU.add)
                    std = pool_rows.tile([1, T], FP32, tag="std", name="std")
                    nc.scalar.activation(std[:], var[:], AF.Sqrt)
                    a_row = pool_rows.tile([1, T], FP32, tag="arow", name="arow")
                    nc.vector.reciprocal(a_row[:], std[:])
                    mu_b = pool_rows.tile([128, T], FP32, tag=f"mub{name}", name=f"mub{name}")
                    nc.gpsimd.partition_broadcast(mu_b[:], mu[:])
                    a_b = pool_rows.tile([128, T], FP32, tag=f"ab{name}", name=f"ab{name}")
                    nc.gpsimd.partition_broadcast(a_b[:], a_row[:])
                    out["a_b"], out["mu_b"] = a_b, mu_b
                    if want_pm:
                        mu_pm = pool_rows.tile([128, NTT], FP32, tag="mupm", name="mupm")
                        a_pm = pool_rows.tile([128, NTT], FP32, tag="apm", name="apm")
                        for tt in range(NTT):
                            tp = pps.tile([128, 128], FP32, tag="tp", name="tp")
                            nc.tensor.transpose(tp[:], mu_b[:, 128 * tt:128 * (tt + 1)], ident[:])
                            nc.vector.tensor_copy(mu_pm[:, tt:tt + 1], tp[:, 0:1])
                            tp2 = pps.tile([128, 128], FP32, tag="tp", name="tp2")
                            nc.tensor.transpose(tp2[:], a_b[:, 128 * tt:128 * (tt + 1)], ident[:])
                            nc.vector.tensor_copy(a_pm[:, tt:tt + 1], tp2[:, 0:1])
                        out["mu_pm"], out["a_pm"] = mu_pm, a_pm
                return out

            _mark("ln1")
            # ============ P1: LN1 ============
            st1 = ln_stats(x_bf, "l1", want_pm=True)
            a1_b, mu1_b = st1["a_b"], st1["mu_b"]
            mu1_pm, a1_pm = st1["mu_pm"], st1["a_pm"]

            _mark("qkv")
            # ============ P2: QKV (on raw x; LN folded into corrections) ============
            qk_sb = [pool_qkv.tile([128, T], BF16, tag=f"qk{ct}", name=f"qk{ct}") for ct in range(16)]
            v_sb = pool_qkv.tile([128, NTT * D], BF16, tag="vsb", name="vsb")
            with tc.tile_pool(name="wqkp" + rp, bufs=6) as pwqk, \
                 tc.tile_pool(name="wvp" + rp, bufs=1) as pwv, \
                 tc.tile_pool(name="p2t" + rp, bufs=3) as pt2, \
                 tc.tile_pool(name="p2ps" + rp, bufs=3, space="PSUM") as pps:
                for ct in range(16 if "qkv" not in skip else 0):
                    h_ps = pps.tile([128, T], FP32, tag="h", name="h")
                    wt = pwqk.tile([128, KT * 128], BF16, tag="wqk", name="wqk")
                    nc.sync.dma_start(wt[:], d_wqk[ct])
                    for r in range(KT):
                        nc.tensor.matmul(h_ps[:], wt[:, 128 * r:128 * (r + 1)], x_bf[r][:],
                                         start=(r == 0), stop=(r == KT - 1))
                    tmp = pt2.tile([128, T], FP32, tag="t", name="t")
                    nc.vector.scalar_tensor_tensor(tmp[:], mu1_b[:], uqk[:, ct:ct + 1],
                                                   h_ps[:], op0=ALU.mult, op1=ALU.add)
                    nc.vector.tensor_mul(tmp[:], tmp[:], a1_b[:])
                    nc.scalar.activation(qk_sb[ct][:], tmp[:], AF.Identity,
                                         bias=cqk[:, ct:ct + 1])
                # ---- send q,k and launch the qk AllToAll first ----
                QSZ = 128 * T                      # 65536 elements
                a1q_in = pool_dram.tile([N_CORES, 2 * QSZ], BF16, name="a1q_in")
                a1q_out = pool_dram.tile([N_CORES, 2 * QSZ], BF16, name="a1q_out")
                for d in range(N_CORES):
                    blk = a1q_in[d, :].rearrange("(p t f) -> p t f", p=128, t=2)
                    nc.sync.dma_start(blk[:, 0, :], qk_sb[d][:])
                    nc.sync.dma_start(blk[:, 1, :], qk_sb[8 + d][:])
                if sim:
                    nc.sync.dma_start(a1q_out[:], a1q_in[:])
                else:
                    nc.gpsimd.collective_compute(
                        "AllToAll", ALU.bypass, replica_groups=[list(range(N_CORES))],
                        ins=[a1q_in.opt()], outs=[a1q_out.opt()])

                _mark("vproj")
                # ---- v projection overlaps the qk AllToAll ----
                wvt = [pwv.tile([128, KT * 512], BF16, tag=f"wv{j}", name=f"wv{j}")
                       for j in range(2)]
                for j in range(2):
                    nc.sync.dma_start(wvt[j][:], d_wv[j])
                for tt in range(NTT if "qkv" not in skip else 0):
                    for j in range(2):
                        hv = pps.tile([128, T], FP32, tag="h", name="h")
                        for r in range(KT):
                            nc.tensor.matmul(hv[:], x_bf[r][:, 128 * tt:128 * (tt + 1)],
                                             wvt[j][:, 512 * r:512 * (r + 1)],
                                             start=(r == 0), stop=(r == KT - 1))
                        tmpv = pt2.tile([128, T], FP32, tag="t", name="t")
                        nc.vector.scalar_tensor_tensor(
                            tmpv[:], uvb[:, 512 * j:512 * (j + 1)], mu1_pm[:, tt:tt + 1],
                            hv[:], op0=ALU.mult, op1=ALU.add)
                        nc.vector.scalar_tensor_tensor(
                            v_sb[:, D * tt + 512 * j:D * tt + 512 * (j + 1)],
                            tmpv[:], a1_pm[:, tt:tt + 1],
                            cvb[:, 512 * j:512 * (j + 1)], op0=ALU.mult, op1=ALU.add)
            cm_xb.__exit__(None, None, None)

            _mark("v_a2a")
            # ============ P3b: v AllToAll ============
            a1v_in = pool_dram.tile([N_CORES, QSZ], BF16, name="a1v_in")
            a1v_out = pool_dram.tile([N_CORES, QSZ], BF16, name="a1v_out")
            for d in range(N_CORES):
                nc.sync.dma_start(
                    a1v_in[d, :].rearrange("(p t f) -> p t f", p=128, t=NTT),
                    v_sb[:].rearrange("p (t f) -> p t f", t=NTT)[:, :, 128 * d:128 * (d + 1)])
            if sim:
                nc.sync.dma_start(a1v_out[:], a1v_in[:])
            else:
                nc.gpsimd.collective_compute(
                    "AllToAll", ALU.bypass, replica_groups=[list(range(N_CORES))],
                    ins=[a1v_in.opt()], outs=[a1v_out.opt()])
            # gather wo|w1|w2 while attention runs (CC queue is idle until the
            # ctx AllToAll; results are first needed at out-proj)
            if sim:
                for g in range(N_CORES):
                    nc.sync.dma_start(
                        d_wfull[g * (WSH // N_CORES):(g + 1) * (WSH // N_CORES)],
                        d_wshb[:])
            else:
                nc.gpsimd.collective_compute(
                    "AllGather", ALU.bypass, replica_groups=[list(range(N_CORES))],
                    ins=[d_wshb.opt()], outs=[d_wfull.opt()])

            qkA = [[pool_att.tile([128, 2 * T], BF16, tag=f"qk{b}_{j}", name=f"qk{b}_{j}")
                    for j in range(4)] for b in range(2)]
            # v tiles carry a ones-column per 64-col (kt%4, hh) group so the
            # probs@v matmul also emits the softmax denominator in psum row 0
            v_att = [[pool_att.tile([128, 520], BF16, tag=f"va{b}_{cb}", name=f"va{b}_{cb}")
                      for cb in range(4)] for b in range(2)]
            for b in range(2):
                for j in range(4):
                    cb = 4 * b + j
                    nc.sync.dma_start(qkA[b][j][:],
                                      a1q_out[cb, :].rearrange("(p f) -> p f", p=128))
                for jcb in range(4):
                    cb = 4 * b + jcb
                    vv = v_att[b][jcb][:].rearrange("p (g c) -> p g c", c=65)
                    nc.vector.memset(vv[:, :, 64:65], 1.0)
                    nc.sync.dma_start(
                        vv[:, :, 0:64],
                        a1v_out[cb, :].rearrange("(p g c) -> p g c", p=128, g=8))
            cm_qkv.__exit__(None, None, None)

            _mark("attn")
            # ============ P4: attention (2 global heads, both batches) ============
            a2_in = pool_dram.tile([N_CORES, QSZ], BF16, name="a2_in")
            a2_out = pool_dram.tile([N_CORES, QSZ], BF16, name="a2_out")
            ctxT = [pool_att.tile([128, S], BF16, tag=f"ctx{b}", name=f"ctx{b}") for b in range(2)]
            with tc.tile_pool(name="sc_ps" + rp, bufs=2, space="PSUM") as psc, \
                 tc.tile_pool(name="cx_ps" + rp, bufs=2, space="PSUM") as pcx, \
                 tc.tile_pool(name="probs" + rp, bufs=6) as ppr, \
                 tc.tile_pool(name="attmp" + rp, bufs=3) as patm:
                for b in range(2 if "attn" not in skip else 0):
                    for qt in range(4):
                        cx = [pcx.tile([65, T], FP32, tag=f"cx{hh}", name=f"cx{hh}")
                              for hh in range(2)]
                        nkt = 4 * qt + 4
                        for kt in range(nkt):
                            # on diagonal tiles, queries [0, 128m) are fully masked:
                            # skip them in scores/exp/ctx entirely
                            m = kt - 4 * qt
                            q0 = 128 * m if m > 0 else 0
                            pr = []
                            for hh in range(2):
                                sc = psc.tile([128, T], FP32, tag=f"sc{hh}", name=f"sc{hh}")
                                nc.tensor.matmul(
                                    sc[:, q0:],
                                    qkA[b][kt // 4][64 * hh:64 * (hh + 1),
                                                    T + 128 * (kt % 4):T + 128 * (kt % 4 + 1)],
                                    qkA[b][qt][64 * hh:64 * (hh + 1), q0:T],
                                    start=True, stop=True)
                                if m >= 0:
                                    nc.vector.tensor_add(
                                        sc[:, q0:], sc[:, q0:],
                                        masks[:, 512 * m + q0:512 * (m + 1)])
                                p = ppr.tile([128, T], BF16, tag=f"p{hh}", name=f"p{hh}")
                                nc.scalar.activation(
                                    p[:, q0:], sc[:, q0:], AF.Exp,
                                    bias=alibi[:, 16 * hh + kt:16 * hh + kt + 1])
                                pr.append(p)
                            for hh in range(2):
                                g = 2 * (kt % 4) + hh
                                nc.tensor.matmul(
                                    cx[hh][0:65, q0:],
                                    v_att[b][kt // 4][:, 65 * g:65 * (g + 1)],
                                    pr[hh][:, q0:], start=(kt == 0), stop=(kt == nkt - 1))
                        for hh in range(2):
                            rec = patm.tile([1, T], FP32, tag=f"rec{hh}", name=f"rec{hh}")
                            nc.vector.reciprocal(rec[:], cx[hh][64:65, :])
                            rec_b = patm.tile([64, T], FP32, tag=f"rb{hh}", name=f"rb{hh}")
                            nc.gpsimd.partition_broadcast(rec_b[:], rec[:])
                            nc.vector.tensor_mul(
                                ctxT[b][64 * hh:64 * (hh + 1), 512 * qt:512 * (qt + 1)],
                                cx[hh][0:64, :], rec_b[:])
                        d = 4 * b + qt
                        nc.sync.dma_start(
                            a2_in[d, :].rearrange("(p f) -> p f", p=128),
                            ctxT[b][:, 512 * qt:512 * (qt + 1)])

            # preload out-proj weights during attention
            wo_sb = [pool_wop.tile([128, KT * 128], BF16, tag=f"wo{dt}", name=f"wo{dt}")
                     for dt in range(KT)]
            for dt in range(KT):
                nc.sync.dma_start(wo_sb[dt][:], d_wo[dt])

            _mark("ctx_a2a")
            # ============ P5: AllToAll #2 (heads -> tokens) ============
            if sim:
                nc.sync.dma_start(a2_out[:], a2_in[:])
            else:
                nc.gpsimd.collective_compute(
                    "AllToAll", ALU.bypass, replica_groups=[list(range(N_CORES))],
                    ins=[a2_in.opt()], outs=[a2_out.opt()])
            ctxf = [pool_ctxf.tile([128, T], BF16, tag=f"cf{r}", name=f"cf{r}") for r in range(KT)]
            for r in range(KT):
                nc.sync.dma_start(ctxf[r][:], a2_out[r, :].rearrange("(p f) -> p f", p=128))
            cm_att.__exit__(None, None, None)

            _mark("outproj")
            # ============ P6: out-proj + residual 1 ============
            src2_f = [pool_s2.tile([128, T], FP32, tag=f"s2f{r}", name=f"s2f{r}") for r in range(KT)]
            src2_bf = [pool_s2.tile([128, T], BF16, tag=f"s2b{r}", name=f"s2b{r}") for r in range(KT)]
            with tc.tile_pool(name="p6ps" + rp, bufs=3, space="PSUM") as pps:
                for dt in range(KT if "oproj" not in skip else 0):
                    op = pps.tile([128, T], FP32, tag="op", name="op")
                    for r in range(KT):
                        nc.tensor.matmul(op[:], wo_sb[dt][:, 128 * r:128 * (r + 1)],
                                         ctxf[r][:], start=(r == 0), stop=(r == KT - 1))
                    nc.vector.scalar_tensor_tensor(src2_f[dt][:], op[:], bo[:, dt:dt + 1],
                                                   x_f[dt][:], op0=ALU.add, op1=ALU.add)
                    nc.scalar.copy(src2_bf[dt][:], src2_f[dt][:])
            cm_wop.__exit__(None, None, None)
            cm_ctxf.__exit__(None, None, None)

            _mark("ln2")
            # ============ P7: LN2 ============
            st2 = ln_stats(src2_bf, "l2", want_pm=False)
            a2_b, mu2_b = st2["a_b"], st2["mu_b"]

            _mark("ffn1")
            # ============ P8: FFN ============
            pw1 = pool_w1p
            with tc.tile_pool(name="h1p" + rp, bufs=1) as ph1, \
                 tc.tile_pool(name="p8t" + rp, bufs=3) as pt8, \
                 tc.tile_pool(name="p8ps" + rp, bufs=3, space="PSUM") as pps:
                h1_sb = [ph1.tile([128, T], BF16, tag=f"h1{ht}", name=f"h1{ht}")
                         for ht in range(DFF // 128)]
                for ht in range(DFF // 128 if "ffn" not in skip else 0):
                    hp = pps.tile([128, T], FP32, tag="hp", name="hp")
                    wt = pw1.tile([128, KT * 128], BF16, tag="w1", name="w1")
                    nc.sync.dma_start(wt[:], d_w1[ht])
                    for r in range(KT):
                        nc.tensor.matmul(hp[:], wt[:, 128 * r:128 * (r + 1)], src2_bf[r][:],
                                         start=(r == 0), stop=(r == KT - 1))
                    tmp = pt8.tile([128, T], FP32, tag="t", name="t")
                    nc.vector.scalar_tensor_tensor(tmp[:], mu2_b[:], u1[:, ht:ht + 1],
                                                   hp[:], op0=ALU.mult, op1=ALU.add)
                    nc.vector.tensor_mul(tmp[:], tmp[:], a2_b[:])
                    nc.scalar.activation(h1_sb[ht][:], tmp[:], AF.Relu,
                                         bias=b1[:, ht:ht + 1])
                _mark("ffn2")
                with tc.tile_pool(name="w2p" + rp, bufs=2) as pw2, \
                     tc.tile_pool(name="p9ps" + rp, bufs=3, space="PSUM") as pps2, \
                     tc.tile_pool(name="outp" + rp, bufs=3) as pout:
                    for dt in range(KT if "ffn" not in skip else 0):
                        hp = pps2.tile([128, T], FP32, tag="hp2", name="hp2")
                        wt = pw2.tile([128, DFF], BF16, tag="w2", name="w2")
                        nc.sync.dma_start(wt[:], d_w2[dt])
                        for hr in range(DFF // 128):
                            nc.tensor.matmul(hp[:], wt[:, 128 * hr:128 * (hr + 1)], h1_sb[hr][:],
                                             start=(hr == 0), stop=(hr == DFF // 128 - 1))
                        of = pout.tile([128, T], FP32, tag="of", name="of")
                        nc.vector.scalar_tensor_tensor(of[:], hp[:], b2[:, dt:dt + 1],
                                                       src2_f[dt][:], op0=ALU.add, op1=ALU.add)
                        nc.sync.dma_start(d_out[128 * dt:128 * (dt + 1), :], of[:])

            cm_w1p.__exit__(None, None, None)


    with tile.TileContext(nc) as tc:
        for rep in range(reps):
            _body(tc, f"r{rep}" if reps > 1 else "")
    nc.compile()
    return nc


def _pm(v):
    """[n*128] -> partition-major [128, n]."""
    n = v.shape[0] // 128
    return np.ascontiguousarray(v.reshape(n, 128).T.astype(np.float32))


def _prep(src, ln1_w, ln1_b, wqkv, bqkv, wo, bo, ln2_w, ln2_b, w1, b1, w2, b2):
    src = np.asarray(src, np.float32)
    ln1_w = np.asarray(ln1_w, np.float32); ln1_b = np.asarray(ln1_b, np.float32)
    wqkv = np.asarray(wqkv, np.float32); bqkv = np.asarray(bqkv, np.float32)
    wo = np.asarray(wo, np.float32); bo = np.asarray(bo, np.float32)
    ln2_w = np.asarray(ln2_w, np.float32); ln2_b = np.asarray(ln2_b, np.float32)
    w1 = np.asarray(w1, np.float32); b1 = np.asarray(b1, np.float32)
    w2 = np.asarray(w2, np.float32); b2 = np.asarray(b2, np.float32)

    wqkv_e = ln1_w[:, None] * wqkv
    bqkv_e = bqkv + ln1_b @ wqkv
    wqkv_e[:, :D] /= np.sqrt(HD)
    bqkv_e[:D] /= np.sqrt(HD)
    u_qkv = wqkv_e.sum(axis=0)
    w1_e = ln2_w[:, None] * w1
    b1_e = b1 + ln2_b @ w1
    u_1 = w1_e.sum(axis=0)

    bf = ml_dtypes.bfloat16
    # layouts: [out_tile, 128 partitions(K within k-tile), k_tile*free] so one
    # DMA per out_tile lands partition-major in SBUF
    wqk_t = np.ascontiguousarray(
        wqkv_e[:, :2 * D].reshape(KT, 128, 16, 128).transpose(2, 1, 0, 3)
        .reshape(16, 128, KT * 128)).astype(bf)
    wv_t = np.ascontiguousarray(
        wqkv_e[:, 2 * D:].reshape(KT, 128, 2, 512).transpose(2, 1, 0, 3)
        .reshape(2, 128, KT * 512)).astype(bf)
    wo_t = np.ascontiguousarray(
        wo.reshape(KT, 128, KT, 128).transpose(2, 1, 0, 3)
        .reshape(KT, 128, KT * 128)).astype(bf)
    w1_t = np.ascontiguousarray(
        w1_e.reshape(KT, 128, 32, 128).transpose(2, 1, 0, 3)
        .reshape(32, 128, KT * 128)).astype(bf)
    w2_t = np.ascontiguousarray(
        w2.reshape(32, 128, KT, 128).transpose(2, 1, 0, 3)
        .reshape(KT, 128, DFF)).astype(bf)

    # u vectors fed NEGATED: corrections compute (mu * (-u)) + H = H - mu*u
    uqk = _pm(-u_qkv[:2 * D]); cqk = _pm(bqkv_e[:2 * D])
    uvb = np.ascontiguousarray((-u_qkv[2 * D:]).astype(bf)).reshape(1, D)
    cvb = np.ascontiguousarray(bqkv_e[2 * D:].astype(bf)).reshape(1, D)
    bo_pm = _pm(bo); u1_pm = _pm(-u_1); b1_pm = _pm(b1_e); b2_pm = _pm(b2)
    ident = np.eye(128, dtype=np.float32)

    f = np.arange(512)[None, :]; p = np.arange(128)[:, None]
    masks = np.concatenate(
        [np.where(f < 128 * m + p, np.float32(NEG), np.float32(0.0)) for m in range(4)],
        axis=1).astype(bf)

    wblob = np.concatenate([wo_t.ravel(), w1_t.ravel(), w2_t.ravel()])
    wsz = wblob.shape[0] // N_CORES
    key = np.arange(S, dtype=np.float32)
    in_maps = []
    for c in range(N_CORES):
        b, j = divmod(c, 4)
        xT = np.ascontiguousarray(src[b, 512 * j:512 * (j + 1), :].T)
        al = np.empty((128, 32), np.float32)
        for hh in range(2):
            slope = 2.0 ** (-(2 * c + hh))
            al[:, 16 * hh:16 * (hh + 1)] = (-slope * key).reshape(16, 128).T
        in_maps.append({
            "xT": xT,
            "wqkt": wqk_t, "wvt": wv_t,
            "wsh": np.ascontiguousarray(wblob[c * wsz:(c + 1) * wsz]),
            "uqk": uqk, "cqk": cqk, "uvb": uvb, "cvb": cvb,
            "bo": bo_pm, "u1": u1_pm, "b1": b1_pm, "b2": b2_pm,
            "alibi": al, "masks": masks, "ident": ident,
        })
    return in_maps


def _assemble(res):
    out = np.empty((B, S, D), np.float32)
    for c in range(N_CORES):
        b, j = divmod(c, 4)
        out[b, 512 * j:512 * (j + 1), :] = res.results[c]["out"].T
    return out


def kernel(**inputs):
    in_maps = _prep(**inputs)
    if "nc" not in _cache:
        _cache["nc"] = _build()
    res = run_bass_kernel_spmd(_cache["nc"], in_maps, core_ids=list(range(N_CORES)))
    return _assemble(res)


def run_traced(inputs, tmpdir):
    """Profiled run (NTFF trace); returns BassKernelResults with exec_time_ns."""
    in_maps = _prep(**inputs)
    if "nc" not in _cache:
        _cache["nc"] = _build()
    return run_bass_kernel_spmd(_cache["nc"], in_maps,
                                core_ids=list(range(N_CORES)),
                                trace=True, tmpdir=tmpdir)



# revision 73
# speedup vs baseline: 401.4912x; 401.4912x over previous
"""AlibiTransformerLayer on 8 TRN2 NeuronCores (Bass/Tile, SPMD).

Sharding:
  - Tokens: core c owns 512 tokens: batch c//4, slice [512*(c%4), 512*(c%4)+512).
    LayerNorms, QKV, out-projection, FFN and residuals run token-sharded.
  - Attention: head-sharded globally: core c handles heads {2c, 2c+1} for BOTH
    batches. AllToAll #1 redistributes q,k,v tokens->heads; AllToAll #2
    redistributes ctx heads->tokens.

Layout: feature-major on chip (features on partitions, tokens on free dim).
Host pre-folds LayerNorm affine into adjacent weights, pre-scales wq by
1/sqrt(hd), and precomputes column sums so projections run on the raw input
with an affine fix-up:  W'x_ln = W'(a*x) + (-mu*a)*colsum(W') + bias.

fp8 (e4m3, DoubleRow) runs the q/k/v projections at ~1.4x PE rate: x is cast
to fp8*16 once, weights are staged fp8 pre-scaled (q *2048, k/v *256), and
the descales ride the existing correction chain (ACT scale immediate, a_pm).
q/k also TRAVEL fp8 (*128 / *16) so the first AllToAll halves; the scores
matmul runs on fp8 at bf16 speed and the exp's scale immediate undoes the
2048x. The ctx return AllToAll is fp8 too: the ALiBi-decay denominator
column is pre-divided by 32 so the reciprocal emits 32/Z and the normalize
multiply writes ctx pre-scaled into fp8 range; the destination casts back.
FFN stays bf16 (fp8 fails the 2e-2 tolerance there).

Softmax runs in scores^T orientation (keys on partitions): the ALiBi bias
-(i+j)*2^-h is separable; -i*s cancels in softmax, and exp(-j*s) is folded
into the V tiles (and the denominator column) as a per-key multiplier with
graceful bf16 underflow — so the exp needs NO per-partition bias and both
heads' scores merge into ONE wide activation per k-tile (HW-measured ~46us
saving vs per-head exps; ACT per-instruction overhead dominates). Causality
is a post-exp 0/1 mask multiply on the bf16 probs (off the exp chain).
The softmax denominator is folded into the probs@v matmul: v SBUF tiles are
[128, 8x65] with the decay column appended per 64-col (kt,head) group, so
one matmul emits ctx rows 0-63 plus the denominator in psum row 64. No
max-subtraction is needed since the j=0 column always contributes O(1).
The attention (b, qt, kt) loop is software-pipelined ACROSS qt/b boundaries:
scores+exp of step k+1 are emitted before ctx of step k, keeping PE busy
while ACT runs and avoiding per-qt pipeline flushes.

Input staging is deduplicated: wo|w1|w2 (identical on all 8 cores) are staged
as a 1/8 shard per core and AllGather'd on device into Shared DRAM while
attention runs (HW-measured faster than direct per-core staging, and all
three gathers must run BEFORE the ctx AllToAll in the CC queue — deferring
w2's gather measured worse); x arrives once as bf16; uvb/cvb ride as [1, D]
DRAM rows DMA-broadcast on device; the output returns bf16.

Engine placement notes (HW-measured): gpsimd/Pool tensor ops are very slow
(~1.7us each) — only partition_broadcast remains there; DVE broadcast-reads
via .to_broadcast() handle the per-head V scaling in one op per tile.
"""

import numpy as np
import ml_dtypes
from contextlib import ExitStack

import concourse.bacc as bacc
import concourse.mybir as mybir
import concourse.tile as tile
from concourse.bass_utils import run_bass_kernel_spmd

FP32 = mybir.dt.float32
BF16 = mybir.dt.bfloat16
FP8 = mybir.dt.float8e4
DR = mybir.MatmulPerfMode.DoubleRow
AF = mybir.ActivationFunctionType
ALU = mybir.AluOpType
# fp8 operand scaling: x is staged *16, wqk q-cols *2048, k-cols *256, wv *256
XS = 16.0
WSQ, WSK, WSV = 2048.0, 256.0, 256.0
# q/k travel through the AllToAll in fp8, scaled *128 / *16; the scores psum
# is then 2048x too big, undone by the exp's scale immediate
SQ8, SK8 = 128.0, 16.0
# ctx returns through its AllToAll in fp8, scaled *32 (via the denominator)
CS8 = 32.0

N_CORES = 8
B, S, D = 2, 2048, 1024
NH, HD = 16, 64
DFF = 4096
EPS = 1e-5
T = 512            # tokens owned per core
NEG = -1e5         # causal mask add
KT = D // 128      # 8 feature k-tiles
NTT = T // 128     # 4 token tiles

_cache = {}


def _build(sim=False, phase_marks=None, reps=1, fake_inputs=False, skip=(), ag=True,
           agsplit=False, exp_merge=True, mask_pool=False, bcast_dma=False,
           vsrc=True):
    def _mark(name):
        if phase_marks is not None:
            phase_marks.append((name, _nc_for_marks.next_id()))
    nc = bacc.Bacc("TRN2", target_bir_lowering=False, debug=False,
                   enable_asserts=True, num_devices=N_CORES)
    _nc_for_marks = nc

    EXT = "Internal" if fake_inputs else "ExternalInput"
    d_xT = nc.dram_tensor("xT", [D, T], BF16, kind=EXT).ap()
    # pre-tiled weights (qkv projections run in fp8 DoubleRow)
    d_wqk = nc.dram_tensor("wqkt", [16, 128, KT * 128], FP8, kind=EXT).ap()
    d_wv = nc.dram_tensor("wvt", [2, 128, KT * 512], FP8, kind=EXT).ap()
    # wo|w1|w2 are identical on every core: stage 1/8 per core, AllGather on
    # device (overlapped with attention) instead of staging 16MB x 8 copies.
    WSH = (KT * 128 * KT * 128) + 2 * (DFF * KT * 128)   # 9437184 elems
    _NWO = KT * 128 * KT * 128
    _NW1 = DFF * KT * 128
    if ag:
        # shard layout per tensor: [wo/8 | w1/8 | w2/8] so the gather can be
        # split — wo+w1 before the ctx AllToAll, w2 after (overlaps ffn1)
        d_wsh = nc.dram_tensor("wsh", [WSH // N_CORES], BF16, kind=EXT).ap()
        d_wshb = nc.dram_tensor("wshb", [WSH // N_CORES], BF16, kind="Internal").ap()
        d_wfo = nc.dram_tensor("wfo", [_NWO], BF16, kind="Internal",
                               addr_space="Shared").ap()
        d_wf1 = nc.dram_tensor("wf1", [_NW1], BF16, kind="Internal",
                               addr_space="Shared").ap()
        d_wf2 = nc.dram_tensor("wf2", [_NW1], BF16, kind="Internal",
                               addr_space="Shared").ap()
        d_wo = d_wfo.rearrange("(a p f) -> a p f", a=KT, p=128)
        d_w1 = d_wf1.rearrange("(a p f) -> a p f", a=DFF // 128, p=128)
        d_w2 = d_wf2.rearrange("(a p f) -> a p f", a=KT, p=128)
    else:
        d_wo = nc.dram_tensor("wot", [KT, 128, KT * 128], BF16, kind=EXT).ap()
        d_w1 = nc.dram_tensor("w1t", [DFF // 128, 128, KT * 128], BF16, kind=EXT).ap()
        d_w2 = nc.dram_tensor("w2t", [KT, 128, DFF], BF16, kind=EXT).ap()
    # per-column constants (partition-major)
    d_uqk = nc.dram_tensor("uqk", [128, 16], FP32, kind="ExternalInput").ap()
    d_cqk = nc.dram_tensor("cqk", [128, 16], FP32, kind=EXT).ap()
    d_u1 = nc.dram_tensor("u1", [128, 32], FP32, kind=EXT).ap()
    d_uvb = nc.dram_tensor("uvb", [1, D], BF16, kind=EXT).ap()
    d_cvb = nc.dram_tensor("cvb", [1, D], BF16, kind=EXT).ap()
    d_bo = nc.dram_tensor("bo", [128, 8], FP32, kind=EXT).ap()
    d_b1 = nc.dram_tensor("b1", [128, 32], FP32, kind=EXT).ap()
    d_b2 = nc.dram_tensor("b2", [128, 8], FP32, kind=EXT).ap()
    # per-key ALiBi decay exp(-slope*j) folded into v (and the denominator
    # column). cols 0-63: source-side layout (tt, h) for v_sb scaling;
    # cols 64-127: dest-side layout (b, cb, g) for the denominator column.
    d_vsc = nc.dram_tensor("vsc", [128, 128], FP32, kind=EXT).ap()
    d_masks = nc.dram_tensor("masks", [128, 2048], BF16, kind=EXT).ap()
    d_ident = nc.dram_tensor("ident", [128, 128], FP32, kind=EXT).ap()
    d_out = nc.dram_tensor("out", [D, T], BF16, kind="ExternalOutput").ap()

    def _body(tc, rp):
        with ExitStack() as ctx:
            pool_const = ctx.enter_context(tc.tile_pool(name="const" + rp, bufs=1))
            pool_rows = ctx.enter_context(tc.tile_pool(name="rows" + rp, bufs=1))
            pool_dram = ctx.enter_context(tc.tile_pool(name="dramp" + rp, bufs=1, space="DRAM"))
            pool_xf = ctx.enter_context(tc.tile_pool(name="xf" + rp, bufs=1))
            pool_s2 = ctx.enter_context(tc.tile_pool(name="s2" + rp, bufs=1))
            # LIFO-scoped pools (closed mid-program, innermost last-opened first)
            cm_w1p = tc.tile_pool(name="w1p" + rp, bufs=6); pool_w1p = cm_w1p.__enter__()
            cm_ctxf = tc.tile_pool(name="ctxf" + rp, bufs=1); pool_ctxf = cm_ctxf.__enter__()
            cm_wop = tc.tile_pool(name="wop" + rp, bufs=1); pool_wop = cm_wop.__enter__()
            cm_att = tc.tile_pool(name="attp" + rp, bufs=1); pool_att = cm_att.__enter__()
            cm_qkv = tc.tile_pool(name="qkvout" + rp, bufs=1); pool_qkv = cm_qkv.__enter__()
            cm_xb = tc.tile_pool(name="xbp" + rp, bufs=1); pool_xb = cm_xb.__enter__()

            if ag:
                nc.sync.dma_start(d_wshb[:], d_wsh[:])
            # x arrives bf16 (long-lived: LN stats + residual); an fp8 copy
            # (scaled by XS, freed after vproj) feeds the DoubleRow projections
            x_bf = [pool_xf.tile([128, T], BF16, tag=f"xb{r}", name=f"xb{r}") for r in range(KT)]
            x_f8t = pool_xb.tile([128, KT * T], FP8, tag="xf8", name="xf8")
            x8v = x_f8t[:].rearrange("p (r t) -> p r t", r=KT)
            for r in range(KT):
                nc.sync.dma_start(x_bf[r][:], d_xT[128 * r:128 * (r + 1), :])
                nc.vector.tensor_scalar_mul(x8v[:, r, :], x_bf[r][:], XS)

            # ---- constants ----
            uqk = pool_const.tile([128, 16], FP32, name="uqk"); nc.sync.dma_start(uqk[:], d_uqk[:])
            cqk = pool_const.tile([128, 16], FP32, name="cqk"); nc.sync.dma_start(cqk[:], d_cqk[:])
            uvb = pool_const.tile([128, D], BF16, name="uvb")
            nc.sync.dma_start(uvb[:], d_uvb.to_broadcast((128, D)))
            cvb = pool_const.tile([128, D], BF16, name="cvb")
            nc.sync.dma_start(cvb[:], d_cvb.to_broadcast((128, D)))
            bo = pool_const.tile([128, 8], FP32, name="bo"); nc.sync.dma_start(bo[:], d_bo[:])
            u1 = pool_const.tile([128, 32], FP32, name="u1"); nc.sync.dma_start(u1[:], d_u1[:])
            b1 = pool_const.tile([128, 32], FP32, name="b1"); nc.sync.dma_start(b1[:], d_b1[:])
            b2 = pool_const.tile([128, 8], FP32, name="b2"); nc.sync.dma_start(b2[:], d_b2[:])
            vsc = pool_const.tile([128, 128], FP32, name="vsc"); nc.sync.dma_start(vsc[:], d_vsc[:])
            masks = pool_const.tile([128, 2048], BF16, name="masks"); nc.sync.dma_start(masks[:], d_masks[:])
            ident = pool_const.tile([128, 128], FP32, name="ident"); nc.sync.dma_start(ident[:], d_ident[:])
            ones_bf = pool_const.tile([128, 1], BF16, name="ones_bf"); nc.vector.memset(ones_bf[:], 1.0)
            # prime the ACT sqrt table while the engines are DMA-bound at startup,
            # so LN1's Sqrt doesn't pay the ~2.7us table load on its critical chain
            prime = pool_const.tile([1, 1], FP32, name="prime")
            nc.vector.memset(prime[:], 1.0)
            nc.scalar.activation(prime[:], prime[:], AF.Sqrt)

            # ---- LN stats helper ----
            def ln_stats(xbf_tiles, name, want_pm=False):
                out = {}
                with tc.tile_pool(name=f"{name}_tmp" + rp, bufs=2) as ptmp, \
                     tc.tile_pool(name=f"{name}_ps" + rp, bufs=1, space="PSUM") as pps:
                    s_ps = pps.tile([1, T], FP32, tag="s", name="s")
                    q_ps = pps.tile([1, T], FP32, tag="q", name="q")
                    for r in range(KT):
                        sq = ptmp.tile([128, T], BF16, tag="sq", name="sq")
                        nc.vector.tensor_mul(sq[:], xbf_tiles[r][:], xbf_tiles[r][:])
                        nc.tensor.matmul(s_ps[:], ones_bf[:], xbf_tiles[r][:],
                                         start=(r == 0), stop=(r == KT - 1))
                        nc.tensor.matmul(q_ps[:], ones_bf[:], sq[:],
                                         start=(r == 0), stop=(r == KT - 1))
                    mu = pool_rows.tile([1, T], FP32, tag="mu", name="mu")
                    nc.scalar.activation(mu[:], s_ps[:], AF.Copy, scale=1.0 / D)
                    m2 = pool_rows.tile([1, T], FP32, tag="m2", name="m2")
                    nc.scalar.activation(m2[:], q_ps[:], AF.Copy, scale=1.0 / D)
                    var = pool_rows.tile([1, T], FP32, tag="var", name="var")
                    nc.vector.scalar_tensor_tensor(var[:], mu[:], -1.0, mu[:],
                                                   op0=ALU.mult, op1=ALU.mult)
                    nc.vector.scalar_tensor_tensor(var[:], m2[:], float(EPS), var[:],
                                                   op0=ALU.add, op1=ALU.add)
                    std = pool_rows.tile([1, T], FP32, tag="std", name="std")
                    nc.scalar.activation(std[:], var[:], AF.Sqrt)
                    a_row = pool_rows.tile([1, T], FP32, tag="arow", name="arow")
                    nc.vector.reciprocal(a_row[:], std[:])
                    mu_b = pool_rows.tile([128, T], FP32, tag=f"mub{name}", name=f"mub{name}")
                    nc.gpsimd.partition_broadcast(mu_b[:], mu[:])
                    a_b = pool_rows.tile([128, T], FP32, tag=f"ab{name}", name=f"ab{name}")
                    nc.gpsimd.partition_broadcast(a_b[:], a_row[:])
                    out["a_b"], out["mu_b"] = a_b, mu_b
                    if want_pm:
                        mu_pm = pool_rows.tile([128, NTT], FP32, tag="mupm", name="mupm")
                        a_pm = pool_rows.tile([128, NTT], FP32, tag="apm", name="apm")
                        for tt in range(NTT):
                            tp = pps.tile([128, 128], FP32, tag="tp", name="tp")
                            nc.tensor.transpose(tp[:], mu_b[:, 128 * tt:128 * (tt + 1)], ident[:])
                            nc.vector.tensor_copy(mu_pm[:, tt:tt + 1], tp[:, 0:1])
                            tp2 = pps.tile([128, 128], FP32, tag="tp", name="tp2")
                            nc.tensor.transpose(tp2[:], a_b[:, 128 * tt:128 * (tt + 1)], ident[:])
                            # fold the fp8 operand descale 1/(XS*WSV) into a_pm
                            # (a_pm is only consumed by the v projection)
                            nc.vector.tensor_scalar_mul(a_pm[:, tt:tt + 1], tp2[:, 0:1],
                                                        1.0 / (XS * WSV))
                        out["mu_pm"], out["a_pm"] = mu_pm, a_pm
                return out

            _mark("ln1")
            # ============ P1: LN1 ============
            st1 = ln_stats(x_bf, "l1", want_pm=True)
            a1_b, mu1_b = st1["a_b"], st1["mu_b"]
            mu1_pm, a1_pm = st1["mu_pm"], st1["a_pm"]

            _mark("qkv")
            # ============ P2: QKV (on raw x; LN folded into corrections) ============
            qk_sb = [pool_qkv.tile([128, T], FP8, tag=f"qk{ct}", name=f"qk{ct}") for ct in range(16)]
            v_sb = pool_qkv.tile([128, NTT * D], BF16, tag="vsb", name="vsb")
            with tc.tile_pool(name="wqkp" + rp, bufs=6) as pwqk, \
                 tc.tile_pool(name="wvp" + rp, bufs=1) as pwv, \
                 tc.tile_pool(name="p2t" + rp, bufs=3) as pt2, \
                 tc.tile_pool(name="p2ps" + rp, bufs=3, space="PSUM") as pps:
                for ct in range(16 if "qkv" not in skip else 0):
                    h_ps = pps.tile([128, T], FP32, tag="h", name="h")
                    wt = pwqk.tile([128, KT * 128], FP8, tag="wqk", name="wqk")
                    nc.sync.dma_start(wt[:], d_wqk[ct])
                    wtv = wt[:].rearrange("p (r m) -> p r m", r=KT)
                    for r in range(KT // 2):
                        nc.tensor.matmul(h_ps[:], wtv[:, 2 * r:2 * r + 2, :],
                                         x8v[:, 2 * r:2 * r + 2, :], perf_mode=DR,
                                         start=(r == 0), stop=(r == KT // 2 - 1))
                    tmp = pt2.tile([128, T], FP32, tag="t", name="t")
                    nc.vector.scalar_tensor_tensor(tmp[:], mu1_b[:], uqk[:, ct:ct + 1],
                                                   h_ps[:], op0=ALU.mult, op1=ALU.add)
                    # ×a on Pool (SBUF-only engine, idle here) to halve DVE load
                    nc.gpsimd.tensor_mul(tmp[:], tmp[:], a1_b[:])
                    # output fp8, pre-scaled by SQ8/SK8 (cqk is host-scaled)
                    nc.scalar.activation(qk_sb[ct][:], tmp[:], AF.Identity,
                                         bias=cqk[:, ct:ct + 1],
                                         scale=(SQ8 if ct < 8 else SK8)
                                         / (XS * (WSQ if ct < 8 else WSK)))
                # ---- send q,k and launch the qk AllToAll first ----
                QSZ = 128 * T                      # 65536 elements
                a1q_in = pool_dram.tile([N_CORES, 2 * QSZ], FP8, name="a1q_in")
                a1q_out = pool_dram.tile([N_CORES, 2 * QSZ], FP8, name="a1q_out")
                for d in range(N_CORES):
                    blk = a1q_in[d, :].rearrange("(p t f) -> p t f", p=128, t=2)
                    nc.sync.dma_start(blk[:, 0, :], qk_sb[d][:])
                    nc.sync.dma_start(blk[:, 1, :], qk_sb[8 + d][:])
                if sim:
                    nc.sync.dma_start(a1q_out[:], a1q_in[:])
                else:
                    nc.gpsimd.collective_compute(
                        "AllToAll", ALU.bypass, replica_groups=[list(range(N_CORES))],
                        ins=[a1q_in.opt()], outs=[a1q_out.opt()])

                _mark("vproj")
                # ---- v projection overlaps the qk AllToAll ----
                wvt = [pwv.tile([128, KT * 512], FP8, tag=f"wv{j}", name=f"wv{j}")
                       for j in range(2)]
                for j in range(2):
                    nc.sync.dma_start(wvt[j][:], d_wv[j])
                for tt in range(NTT if "qkv" not in skip else 0):
                    for j in range(2):
                        hv = pps.tile([128, T], FP32, tag="h", name="h")
                        wvv = wvt[j][:].rearrange("p (r f) -> p r f", r=KT)
                        for r in range(KT // 2):
                            nc.tensor.matmul(hv[:],
                                             x8v[:, 2 * r:2 * r + 2, 128 * tt:128 * (tt + 1)],
                                             wvv[:, 2 * r:2 * r + 2, :], perf_mode=DR,
                                             start=(r == 0), stop=(r == KT // 2 - 1))
                        tmpv = pt2.tile([128, T], FP32, tag="t", name="t")
                        nc.vector.scalar_tensor_tensor(
                            tmpv[:], uvb[:, 512 * j:512 * (j + 1)], mu1_pm[:, tt:tt + 1],
                            hv[:], op0=ALU.mult, op1=ALU.add)
                        nc.vector.scalar_tensor_tensor(
                            v_sb[:, D * tt + 512 * j:D * tt + 512 * (j + 1)],
                            tmpv[:], a1_pm[:, tt:tt + 1],
                            cvb[:, 512 * j:512 * (j + 1)], op0=ALU.mult, op1=ALU.add)
                        if vsrc:
                            # fold the ALiBi decay into v at the source (one
                            # broadcast-multiply, overlaps the qk AllToAll)
                            vslc = v_sb[:, D * tt + 512 * j:D * tt + 512 * (j + 1)] \
                                .rearrange("p (h c) -> p h c", c=64)
                            nc.vector.tensor_mul(
                                vslc[:, :, :], vslc[:, :, :],
                                vsc[:, 16 * tt + 8 * j:16 * tt + 8 * j + 8]
                                .rearrange("p (h c) -> p h c", c=1)
                                .to_broadcast((128, 8, 64)))

            cm_xb.__exit__(None, None, None)

            _mark("v_a2a")
            # ============ P3b: v AllToAll ============
            a1v_in = pool_dram.tile([N_CORES, QSZ], BF16, name="a1v_in")
            a1v_out = pool_dram.tile([N_CORES, QSZ], BF16, name="a1v_out")
            for d in range(N_CORES):
                nc.sync.dma_start(
                    a1v_in[d, :].rearrange("(p t f) -> p t f", p=128, t=NTT),
                    v_sb[:].rearrange("p (t f) -> p t f", t=NTT)[:, :, 128 * d:128 * (d + 1)])
            if sim:
                nc.sync.dma_start(a1v_out[:], a1v_in[:])
            else:
                nc.gpsimd.collective_compute(
                    "AllToAll", ALU.bypass, replica_groups=[list(range(N_CORES))],
                    ins=[a1v_in.opt()], outs=[a1v_out.opt()])
            # gather wo|w1 while attention runs (CC queue is idle until the
            # ctx AllToAll); w2's gather is issued after the ctx AllToAll so
            # that a2a isn't queued behind the whole 16MB
            _SO, _S1 = _NWO // N_CORES, _NW1 // N_CORES
            if ag:
                if sim:
                    for g in range(N_CORES):
                        nc.sync.dma_start(d_wfo[g * _SO:(g + 1) * _SO], d_wshb[0:_SO])
                        nc.sync.dma_start(d_wf1[g * _S1:(g + 1) * _S1],
                                          d_wshb[_SO:_SO + _S1])
                        if not agsplit:
                            nc.sync.dma_start(d_wf2[g * _S1:(g + 1) * _S1],
                                              d_wshb[_SO + _S1:])
                else:
                    nc.gpsimd.collective_compute(
                        "AllGather", ALU.bypass, replica_groups=[list(range(N_CORES))],
                        ins=[d_wshb[0:_SO].opt()], outs=[d_wfo.opt()])
                    nc.gpsimd.collective_compute(
                        "AllGather", ALU.bypass, replica_groups=[list(range(N_CORES))],
                        ins=[d_wshb[_SO:_SO + _S1].opt()], outs=[d_wf1.opt()])
                    if not agsplit:
                        nc.gpsimd.collective_compute(
                            "AllGather", ALU.bypass,
                            replica_groups=[list(range(N_CORES))],
                            ins=[d_wshb[_SO + _S1:].opt()], outs=[d_wf2.opt()])

            qkA = [[pool_att.tile([128, 2 * T], FP8, tag=f"qk{b}_{j}", name=f"qk{b}_{j}")
                    for j in range(4)] for b in range(2)]
            # v tiles carry a ones-column per 64-col (kt%4, hh) group so the
            # probs@v matmul also emits the softmax denominator in psum row 0
            v_att = [[pool_att.tile([128, 520], BF16, tag=f"va{b}_{cb}", name=f"va{b}_{cb}")
                      for cb in range(4)] for b in range(2)]
            for b in range(2):
                for j in range(4):
                    cb = 4 * b + j
                    nc.sync.dma_start(qkA[b][j][:],
                                      a1q_out[cb, :].rearrange("(p f) -> p f", p=128))
                for jcb in range(4):
                    cb = 4 * b + jcb
                    vv = v_att[b][jcb][:].rearrange("p (g c) -> p g c", c=65)
                    nc.sync.dma_start(
                        vv[:, :, 0:64],
                        a1v_out[cb, :].rearrange("(p g c) -> p g c", p=128, g=8))
                    vcol = 64 + 32 * b + 8 * jcb
                    if vsrc:
                        # v arrives pre-scaled; denominator column = decay/CS8,
                        # so the reciprocal emits CS8/Z and the normalize mul
                        # writes ctx pre-scaled for the fp8 return AllToAll
                        nc.vector.tensor_scalar_mul(
                            vv[:, :, 64:65],
                            vsc[:, vcol:vcol + 8].rearrange("p (g c) -> p g c", c=1),
                            1.0 / CS8)
                    else:
                        nc.vector.memset(vv[:, :, 64:65], 1.0)
                        # fold the ALiBi decay into v AND the denominator column
                        nc.vector.tensor_mul(
                            vv[:, :, :], vv[:, :, :],
                            vsc[:, vcol:vcol + 8].rearrange("p (g c) -> p g c", c=1)
                            .to_broadcast((128, 8, 65)))
            cm_qkv.__exit__(None, None, None)

            _mark("attn")
            # ============ P4: attention (2 global heads, both batches) ============
            a2_in = pool_dram.tile([N_CORES, QSZ], FP8, name="a2_in")
            a2_out = pool_dram.tile([N_CORES, QSZ], FP8, name="a2_out")
            ctxT = [pool_att.tile([128, S], FP8, tag=f"ctx{b}", name=f"ctx{b}") for b in range(2)]
            with tc.tile_pool(name="sc_ps" + rp, bufs=2, space="PSUM") as psc, \
                 tc.tile_pool(name="cx_ps" + rp, bufs=2, space="PSUM") as pcx, \
                 tc.tile_pool(name="probs" + rp, bufs=4) as ppr, \
                 tc.tile_pool(name="attmp" + rp, bufs=3) as patm:
                cxs = {}

                def _scores(b, qt, kt):
                    # scores^T for both heads into one 2-bank psum tile, one
                    # merged exp (no bias: ALiBi decay is pre-folded into v;
                    # causality is a post-exp 0/1 multiply on the bf16 probs)
                    m = kt - 4 * qt
                    q0 = 128 * m if m > 0 else 0
                    sc = psc.tile([128, 2 * T], FP32, tag="sc", name="sc")
                    scv = sc[:].rearrange("p (h q) -> p h q", h=2)
                    for hh in range(2):
                        nc.tensor.matmul(
                            scv[:, hh, q0:],
                            qkA[b][kt // 4][64 * hh:64 * (hh + 1),
                                            T + 128 * (kt % 4):T + 128 * (kt % 4 + 1)],
                            qkA[b][qt][64 * hh:64 * (hh + 1), q0:T],
                            start=True, stop=True)
                    p = ppr.tile([128, 2 * T], BF16, tag="p", name="p")
                    pv = p[:].rearrange("p (h q) -> p h q", h=2)
                    if exp_merge:
                        nc.scalar.activation(pv[:, :, q0:], scv[:, :, q0:], AF.Exp,
                                             scale=1.0 / (SQ8 * SK8))
                    else:
                        for hh in range(2):
                            nc.scalar.activation(pv[:, hh, q0:], scv[:, hh, q0:], AF.Exp,
                                                 scale=1.0 / (SQ8 * SK8))
                    if m >= 0:
                        eng = nc.gpsimd if mask_pool else nc.vector
                        for hh in range(2):
                            eng.tensor_mul(
                                pv[:, hh, q0:], pv[:, hh, q0:],
                                masks[:, 512 * m + q0:512 * (m + 1)])
                    return pv, q0

                def _ctx(b, qt, kt, pv, q0):
                    cx, nkt = cxs[(b, qt)], 4 * qt + 4
                    for hh in range(2):
                        g = 2 * (kt % 4) + hh
                        nc.tensor.matmul(
                            cx[hh][0:65, q0:],
                            v_att[b][kt // 4][:, 65 * g:65 * (g + 1)],
                            pv[:, hh, q0:], start=(kt == 0), stop=(kt == nkt - 1))
                    if kt == nkt - 1:
                        for hh in range(2):
                            rec = patm.tile([1, T], FP32, tag=f"rec{hh}", name=f"rec{hh}")
                            nc.vector.reciprocal(rec[:], cx[hh][64:65, :])
                            rec_b = patm.tile([64, T], FP32, tag=f"rb{hh}", name=f"rb{hh}")
                            if bcast_dma:
                                nc.sync.dma_start(rec_b[:], rec[:].to_broadcast((64, T)))
                            else:
                                nc.gpsimd.partition_broadcast(rec_b[:], rec[:])
                            nc.vector.tensor_mul(
                                ctxT[b][64 * hh:64 * (hh + 1), 512 * qt:512 * (qt + 1)],
                                cx[hh][0:64, :], rec_b[:])
                        nc.sync.dma_start(
                            a2_in[4 * b + qt, :].rearrange("(p f) -> p f", p=128),
                            ctxT[b][:, 512 * qt:512 * (qt + 1)])

                # software pipeline across all (b, qt, kt): scores/exp of the
                # next step are emitted before ctx of the current one so PE has
                # work while ACT runs the exp — including across qt/b boundaries
                steps = [(b, qt, kt)
                         for b in range(2 if "attn" not in skip else 0)
                         for qt in range(4) for kt in range(4 * qt + 4)]
                prev = None
                for (b, qt, kt) in steps:
                    if kt == 0:
                        cxs[(b, qt)] = [
                            pcx.tile([65, T], FP32, tag=f"cx{hh}", name=f"cx{hh}")
                            for hh in range(2)]
                    cur = (b, qt, kt, *_scores(b, qt, kt))
                    if prev is not None:
                        _ctx(*prev)
                    prev = cur
                if prev is not None:
                    _ctx(*prev)

            # preload out-proj weights during attention
            wo_sb = [pool_wop.tile([128, KT * 128], BF16, tag=f"wo{dt}", name=f"wo{dt}")
                     for dt in range(KT)]
            for dt in range(KT):
                nc.sync.dma_start(wo_sb[dt][:], d_wo[dt])

            _mark("ctx_a2a")
            # ============ P5: AllToAll #2 (heads -> tokens) ============
            if sim:
                nc.sync.dma_start(a2_out[:], a2_in[:])
            else:
                nc.gpsimd.collective_compute(
                    "AllToAll", ALU.bypass, replica_groups=[list(range(N_CORES))],
                    ins=[a2_in.opt()], outs=[a2_out.opt()])
            if ag and agsplit:
                # w2's gather rides behind the ctx AllToAll; first needed at ffn2
                if sim:
                    for g in range(N_CORES):
                        nc.sync.dma_start(d_wf2[g * _S1:(g + 1) * _S1],
                                          d_wshb[_SO + _S1:])
                else:
                    nc.gpsimd.collective_compute(
                        "AllGather", ALU.bypass, replica_groups=[list(range(N_CORES))],
                        ins=[d_wshb[_SO + _S1:].opt()], outs=[d_wf2.opt()])
            ctxf = [pool_ctxf.tile([128, T], BF16, tag=f"cf{r}", name=f"cf{r}") for r in range(KT)]
            ctxf8 = [pool_ctxf.tile([128, T], FP8, tag=f"c8{r}", name=f"c8{r}") for r in range(KT)]
            for r in range(KT):
                nc.sync.dma_start(ctxf8[r][:], a2_out[r, :].rearrange("(p f) -> p f", p=128))
                nc.vector.tensor_scalar_mul(ctxf[r][:], ctxf8[r][:], 1.0 / CS8)
            cm_att.__exit__(None, None, None)

            _mark("outproj")
            # ============ P6: out-proj + residual 1 ============
            src2_f = [pool_s2.tile([128, T], FP32, tag=f"s2f{r}", name=f"s2f{r}") for r in range(KT)]
            src2_bf = [pool_s2.tile([128, T], BF16, tag=f"s2b{r}", name=f"s2b{r}") for r in range(KT)]
            with tc.tile_pool(name="p6ps" + rp, bufs=3, space="PSUM") as pps:
                for dt in range(KT if "oproj" not in skip else 0):
                    op = pps.tile([128, T], FP32, tag="op", name="op")
                    for r in range(KT):
                        nc.tensor.matmul(op[:], wo_sb[dt][:, 128 * r:128 * (r + 1)],
                                         ctxf[r][:], start=(r == 0), stop=(r == KT - 1))
                    nc.vector.scalar_tensor_tensor(src2_f[dt][:], op[:], bo[:, dt:dt + 1],
                                                   x_bf[dt][:], op0=ALU.add, op1=ALU.add)
                    nc.scalar.copy(src2_bf[dt][:], src2_f[dt][:])
            cm_wop.__exit__(None, None, None)
            cm_ctxf.__exit__(None, None, None)

            _mark("ln2")
            # ============ P7: LN2 ============
            st2 = ln_stats(src2_bf, "l2", want_pm=False)
            a2_b, mu2_b = st2["a_b"], st2["mu_b"]

            _mark("ffn1")
            # ============ P8: FFN ============
            pw1 = pool_w1p
            with tc.tile_pool(name="h1p" + rp, bufs=1) as ph1, \
                 tc.tile_pool(name="p8t" + rp, bufs=3) as pt8, \
                 tc.tile_pool(name="p8ps" + rp, bufs=3, space="PSUM") as pps:
                h1_sb = [ph1.tile([128, T], BF16, tag=f"h1{ht}", name=f"h1{ht}")
                         for ht in range(DFF // 128)]
                for ht in range(DFF // 128 if "ffn" not in skip else 0):
                    hp = pps.tile([128, T], FP32, tag="hp", name="hp")
                    wt = pw1.tile([128, KT * 128], BF16, tag="w1", name="w1")
                    nc.sync.dma_start(wt[:], d_w1[ht])
                    for r in range(KT):
                        nc.tensor.matmul(hp[:], wt[:, 128 * r:128 * (r + 1)], src2_bf[r][:],
                                         start=(r == 0), stop=(r == KT - 1))
                    tmp = pt8.tile([128, T], FP32, tag="t", name="t")
                    nc.vector.scalar_tensor_tensor(tmp[:], mu2_b[:], u1[:, ht:ht + 1],
                                                   hp[:], op0=ALU.mult, op1=ALU.add)
                    nc.vector.tensor_mul(tmp[:], tmp[:], a2_b[:])
                    nc.scalar.activation(h1_sb[ht][:], tmp[:], AF.Relu,
                                         bias=b1[:, ht:ht + 1])
                _mark("ffn2")
                with tc.tile_pool(name="w2p" + rp, bufs=2) as pw2, \
                     tc.tile_pool(name="p9ps" + rp, bufs=3, space="PSUM") as pps2, \
                     tc.tile_pool(name="outp" + rp, bufs=3) as pout:
                    for dt in range(KT if "ffn" not in skip else 0):
                        hp = pps2.tile([128, T], FP32, tag="hp2", name="hp2")
                        wt = pw2.tile([128, DFF], BF16, tag="w2", name="w2")
                        nc.sync.dma_start(wt[:], d_w2[dt])
                        for hr in range(DFF // 128):
                            nc.tensor.matmul(hp[:], wt[:, 128 * hr:128 * (hr + 1)], h1_sb[hr][:],
                                             start=(hr == 0), stop=(hr == DFF // 128 - 1))
                        of = pout.tile([128, T], BF16, tag="of", name="of")
                        nc.vector.scalar_tensor_tensor(of[:], hp[:], b2[:, dt:dt + 1],
                                                       src2_f[dt][:], op0=ALU.add, op1=ALU.add)
                        nc.sync.dma_start(d_out[128 * dt:128 * (dt + 1), :], of[:])

            cm_w1p.__exit__(None, None, None)


    with tile.TileContext(nc) as tc:
        for rep in range(reps):
            _body(tc, f"r{rep}" if reps > 1 else "")
    nc.compile()
    return nc


def _pm(v):
    """[n*128] -> partition-major [128, n]."""
    n = v.shape[0] // 128
    return np.ascontiguousarray(v.reshape(n, 128).T.astype(np.float32))


def _prep(src, ln1_w, ln1_b, wqkv, bqkv, wo, bo, ln2_w, ln2_b, w1, b1, w2, b2,
          ag=True):
    src = np.asarray(src, np.float32)
    ln1_w = np.asarray(ln1_w, np.float32); ln1_b = np.asarray(ln1_b, np.float32)
    wqkv = np.asarray(wqkv, np.float32); bqkv = np.asarray(bqkv, np.float32)
    wo = np.asarray(wo, np.float32); bo = np.asarray(bo, np.float32)
    ln2_w = np.asarray(ln2_w, np.float32); ln2_b = np.asarray(ln2_b, np.float32)
    w1 = np.asarray(w1, np.float32); b1 = np.asarray(b1, np.float32)
    w2 = np.asarray(w2, np.float32); b2 = np.asarray(b2, np.float32)

    wqkv_e = ln1_w[:, None] * wqkv
    bqkv_e = bqkv + ln1_b @ wqkv
    wqkv_e[:, :D] /= np.sqrt(HD)
    bqkv_e[:D] /= np.sqrt(HD)
    u_qkv = wqkv_e.sum(axis=0)
    w1_e = ln2_w[:, None] * w1
    b1_e = b1 + ln2_b @ w1
    u_1 = w1_e.sum(axis=0)

    bf = ml_dtypes.bfloat16
    f8 = ml_dtypes.float8_e4m3
    # layouts: [out_tile, 128 partitions(K within k-tile), k_tile*free] so one
    # DMA per out_tile lands partition-major in SBUF.
    # q/k/v weights are staged fp8 (e4m3, clipped to TRN's ±240), pre-scaled to
    # center the distribution; the descale rides the existing correction chain
    wqkv_s = wqkv_e[:, :2 * D] * np.concatenate(
        [np.full(D, WSQ, np.float32), np.full(D, WSK, np.float32)])
    wqk_t = np.clip(np.ascontiguousarray(
        wqkv_s.reshape(KT, 128, 16, 128).transpose(2, 1, 0, 3)
        .reshape(16, 128, KT * 128)), -240, 240).astype(f8)
    wv_t = np.clip(np.ascontiguousarray(
        (wqkv_e[:, 2 * D:] * WSV).reshape(KT, 128, 2, 512).transpose(2, 1, 0, 3)
        .reshape(2, 128, KT * 512)), -240, 240).astype(f8)
    wo_t = np.ascontiguousarray(
        wo.reshape(KT, 128, KT, 128).transpose(2, 1, 0, 3)
        .reshape(KT, 128, KT * 128)).astype(bf)
    w1_t = np.ascontiguousarray(
        w1_e.reshape(KT, 128, 32, 128).transpose(2, 1, 0, 3)
        .reshape(32, 128, KT * 128)).astype(bf)
    w2_t = np.ascontiguousarray(
        w2.reshape(32, 128, KT, 128).transpose(2, 1, 0, 3)
        .reshape(KT, 128, DFF)).astype(bf)

    # u vectors fed NEGATED and pre-scaled by the fp8 operand scales:
    # corrections compute (mu * (-s*u)) + s*H = s*(H - mu*u)
    uqk = _pm(-u_qkv[:2 * D] * XS * np.concatenate(
        [np.full(D, WSQ, np.float32), np.full(D, WSK, np.float32)]))
    # q/k biases ride pre-scaled by the fp8 transport scales SQ8/SK8
    cqk = _pm(bqkv_e[:2 * D] * np.concatenate(
        [np.full(D, SQ8, np.float32), np.full(D, SK8, np.float32)]))
    uvb = np.ascontiguousarray((-u_qkv[2 * D:] * XS * WSV).astype(bf)).reshape(1, D)
    cvb = np.ascontiguousarray(bqkv_e[2 * D:].astype(bf)).reshape(1, D)
    bo_pm = _pm(bo); u1_pm = _pm(-u_1); b1_pm = _pm(b1_e); b2_pm = _pm(b2)
    ident = np.eye(128, dtype=np.float32)

    # 0/1 keep-masks: causality is applied as a post-exp multiply on the probs
    f = np.arange(512)[None, :]; p = np.arange(128)[:, None]
    masks = np.concatenate(
        [np.where(f < 128 * m + p, np.float32(0.0), np.float32(1.0)) for m in range(4)],
        axis=1).astype(bf)

    # per-tensor shards: wsh_c = [wo/8 | w1/8 | w2/8] (split AllGathers)
    _shards = [np.concatenate([w.ravel().reshape(N_CORES, -1)[c]
                               for w in (wo_t, w1_t, w2_t)])
               for c in range(N_CORES)]
    in_maps = []
    for c in range(N_CORES):
        b, j = divmod(c, 4)
        xT = np.ascontiguousarray(src[b, 512 * j:512 * (j + 1), :].T.astype(bf))
        # per-key ALiBi decay exp(-slope * j_key).
        # cols 0-63 (source side, scales v_sb): col = 16*tt + h for ALL heads h,
        #   key j = 512*(c%4) + 128*tt + p.
        # cols 64-127 (dest side, denominator column): col = 64+32*b+8*jcb+g,
        #   g = 2*tt + hh (this core's heads 2c+hh), j = 512*jcb + 128*tt + p.
        vs = np.empty((128, 128), np.float32)
        pcol = np.arange(128, dtype=np.float64)
        for tt in range(4):
            for h in range(16):
                jkey = 512 * (c % 4) + 128 * tt + pcol
                vs[:, 16 * tt + h] = np.exp(-(2.0 ** (-h)) * jkey).astype(np.float32)
        for vb in range(2):
            for jcb in range(4):
                for tt in range(4):
                    for hh in range(2):
                        slope = 2.0 ** (-(2 * c + hh))
                        jkey = 512 * jcb + 128 * tt + pcol
                        vs[:, 64 + 32 * vb + 8 * jcb + 2 * tt + hh] = np.exp(
                            -slope * jkey).astype(np.float32)
        im = {
            "xT": xT,
            "wqkt": wqk_t, "wvt": wv_t,
            "uqk": uqk, "cqk": cqk, "uvb": uvb, "cvb": cvb,
            "bo": bo_pm, "u1": u1_pm, "b1": b1_pm, "b2": b2_pm,
            "vsc": vs, "masks": masks, "ident": ident,
        }
        if ag:
            im["wsh"] = _shards[c]
        else:
            im["wot"], im["w1t"], im["w2t"] = wo_t, w1_t, w2_t
        in_maps.append(im)
    return in_maps


def _assemble(res):
    out = np.empty((B, S, D), np.float32)
    for c in range(N_CORES):
        b, j = divmod(c, 4)
        out[b, 512 * j:512 * (j + 1), :] = res.results[c]["out"].T.astype(np.float32)
    return out


def kernel(**inputs):
    in_maps = _prep(**inputs)
    if "nc" not in _cache:
        _cache["nc"] = _build()
    res = run_bass_kernel_spmd(_cache["nc"], in_maps, core_ids=list(range(N_CORES)))
    return _assemble(res)


def run_traced(inputs, tmpdir):
    """Profiled run (NTFF trace); returns BassKernelResults with exec_time_ns."""
    in_maps = _prep(**inputs)
    if "nc" not in _cache:
        _cache["nc"] = _build()
    return run_bass_kernel_spmd(_cache["nc"], in_maps,
                                core_ids=list(range(N_CORES)),
                                trace=True, tmpdir=tmpdir)

